# revision 22
# baseline (speedup 1.0000x reference)
"""
Causal self-attention (single head) on 8 trn2 NeuronCores.

Problem: x[4, 2048, 1024], Wq/Wk/Wv[1024, 1024] (torch Linear layout [d_out, d_in]).
    q/k/v = x @ W.T ; out = softmax(mask(q k^T) / 32) @ v

Sharding (no collectives, uniform SPMD program):
  core c -> batch b = c // 2, role r = c % 2.
  Both cores of a pair compute K/V projections for the full 2048-row
  sequence of their batch (duplicated work, avoids cross-core comms).
  Query rows are split between the pair in 4 i-blocks of 256 rows with
  per-slot padded causal extents JT_SLOTS = [4, 8, 12, 16] identical for
  both roles; causality inside the padded slots is enforced with a
  per-core "delta" input (keep iff jj - ii <= delta).

Precision plan (error gate is 2e-2 relative; measured 1.45e-2):
  fp16 is the working dtype (all |values| well inside fp16 range).
  K-projection runs as fp8e4m3 DoubleRow matmuls (2 weights/PE cell,
  2x MACs/cycle, measured 222ns per 256-deep x 512-wide MM vs 2x217ns
  in fp16); q/k are stored fp16 and scores run fp16 (fp8 q/k stores +
  fp8 scores measured 2.4e-2 -- over the gate). V path stays fp16.
  The output DMAs as fp16 and is widened to fp32 on the host.

Everything is SBUF-resident (~170 KB of the 208 KB/partition): x in
fp16 + fp8, all three weights, kT16/qT16, and the 16 v tiles. No DRAM
spills; the only DMA is inputs in (~11 MB) and the output (2 MB).
Inputs are host-packed to the SBUF partition-major layout so each load
is 128 contiguous runs (sliced d-major views cost ~11ns/descriptor x
1024 descriptors of engine issue time per load, and the transfers ran
at half bandwidth). The two tensors the first Q psum groups read ship
as their own contiguous tensors and load first; PE warm-up matmuls on
a memset tile bridge the DMA cold start (~6.5us framework preamble +
~11us first-wave DMA) while holding the HAM clock gate at 8/8.

Phase B is software-pipelined: scores(t+1) is issued to the PE before
ctx(t), so the activation-engine exp(t) latency hides behind the next
scores matmul and the PE never idles. Measured PE idle inside the
kernel body: ~1-4us total.
"""

import sys

for _p in ("/opt/trn_rl_repo", "/root/.axon_site/_ro/trn_rl_repo"):
    if _p not in sys.path:
        sys.path.append(_p)

import numpy as np
import ml_dtypes

import concourse.bass as bass
import concourse.mybir as mybir
import concourse.tile as tile
from concourse import bacc
from concourse.bass_utils import run_bass_kernel_spmd
import concourse.bass_utils as _bu

# NOTE: walrus --enable-ldw-opt stays at its default (false): the measured
# back-to-back MM rate is full speed without it for fp16/fp8 (216.5 ns per
# 512-row MM), and enabling it rejects DoubleRow LDWEIGHTS instructions.

F32 = mybir.dt.float32
F16 = mybir.dt.float16
F8 = mybir.dt.float8e4
DR = mybir.MatmulPerfMode.DoubleRow

B, S, D = 4, 2048, 1024
P = 128
ND = D // P          # 8 d-tiles (projection contraction)
NO = D // P          # 8 o-tiles
IB = 256             # i-block (query block) rows
N_IB = 4
JT_SLOTS = [4, 8, 12, 16]
ROLE_STARTS = {
    0: [0, 768, 1024, 1792],
    1: [256, 512, 1280, 1536],
}
N_CORES = 8
N_WARM = 30


def _mm(nc, out, lhsT, rhs, start, stop, perf_mode=None):
    nc.tensor.matmul(out, lhsT, rhs, start=start, stop=stop,
                     perf_mode=perf_mode)


def build_program():
    nc = bacc.Bacc(
        "TRN2",
        target_bir_lowering=False,
        debug=False,
        enable_asserts=False,
        num_devices=N_CORES,
    )
    # Inputs are host-packed to the SBUF partition-major layout
    # [P, ND*C]: each DMA is 128 contiguous 16KB-ish runs instead of 1024
    # 1KB runs -- descriptor generation (~11ns each) was costing 5-11us of
    # engine issue time per load the d-major way.
    xT16_in = nc.dram_tensor("xT16", [P, ND * S], F16, kind="ExternalInput").ap()
    x8_in = nc.dram_tensor("x8", [P, ND * S], F8, kind="ExternalInput").ap()
    # wq/xq ship as per-chunk contiguous tensors: the first Q psum groups
    # need only wq half 0 + xq chunk 0 (2MB), and a fully-contiguous
    # [P, 8K] DMA has 128 descriptors vs 1024 for a sliced view.
    # Q runs half-fp8: contraction d-tiles 0..3 as fp8 DoubleRow, 4..7 in
    # fp16 (full-fp8 q would push the scores error over the 2e-2 gate; the
    # half split lands ~1.9e-2 and shaves 7us off the Q projection).
    xq_ins = [nc.dram_tensor(f"xqT{i}", [P, 2 * 512], F16,
                             kind="ExternalInput").ap() for i in range(2)]
    xq8_ins = [nc.dram_tensor(f"xq8T{i}", [P, 6 * 512], F8,
                              kind="ExternalInput").ap() for i in range(2)]
    wq_ins = [nc.dram_tensor(f"wqT{i}", [P, 2 * P], F16,
                             kind="ExternalInput").ap() for i in range(8)]
    wq8_ins = [nc.dram_tensor(f"wq8T{i}", [P, 6 * P], F8,
                              kind="ExternalInput").ap() for i in range(8)]
    wk_in = nc.dram_tensor("wkT", [P, ND * D], F8, kind="ExternalInput").ap()
    wv_in = nc.dram_tensor("wvT", [P, ND * D], F16, kind="ExternalInput").ap()
    t0_in = nc.dram_tensor("t0", [P, IB], F16, kind="ExternalInput").ap()
    delta_in = nc.dram_tensor("delta", [P, 16], F16, kind="ExternalInput").ap()
    ones_in = nc.dram_tensor("ones", [P, 2], F16, kind="ExternalInput").ap()
    out = nc.dram_tensor("out", [N_IB * IB, D], F16, kind="ExternalOutput").ap()

    scale = 1.0 / 32.0  # 1/sqrt(d_v)

    def packed(ap2d, c):
        # [P, ND*C] host-packed DRAM view -> [P, ND, C] 3D AP
        return ap2d.rearrange("p (nd c) -> p nd c", c=c)

    with tile.TileContext(nc) as tc:
        with tc.tile_pool(name="res", bufs=1) as rp:
            # ---- constants first (warm-up stationary/moving data) ----
            t0_t = rp.tile([P, IB], F16, tag="t0")
            nc.gpsimd.dma_start(t0_t[:], t0_in[:])
            delta_t = rp.tile([P, 16], F16, tag="delta")
            nc.gpsimd.dma_start(delta_t[:], delta_in[:])
            ones_t = rp.tile([P, 2], F16, tag="ones")
            nc.gpsimd.dma_start(ones_t[:], ones_in[:])

            # ---- resident tensors + their loads ----
            # Two DMA queues, each FIFO-serialized in consumption order so
            # the Q-projection inputs get the full HBM bandwidth first
            # (concurrent queues split it and delayed Q's start by ~11us).
            # First wave is only the 2MB the first Q psum groups need
            # (wq o-half 0 + xq chunk 0), so Q starts ~6us earlier; the
            # rest streams behind it in consumption order, per-queue FIFO.
            xq16 = [rp.tile([P, 2, 512], F16, tag=f"xq16_{i}",
                            name=f"xq16_{i}") for i in range(2)]
            xq8 = [rp.tile([P, 6, 512], F8, tag=f"xq8_{i}",
                           name=f"xq8_{i}") for i in range(2)]
            wq16 = [rp.tile([P, 2, P], F16, tag=f"wq16_{i}",
                            name=f"wq16_{i}") for i in range(8)]
            wq8 = [rp.tile([P, 6, P], F8, tag=f"wq8_{i}",
                           name=f"wq8_{i}") for i in range(8)]
            nc.scalar.dma_start(xq8[0][:], packed(xq8_ins[0], 512))
            nc.scalar.dma_start(xq16[0][:], packed(xq_ins[0], 512))
            for o in range(8):
                nc.sync.dma_start(wq8[o][:], packed(wq8_ins[o], P))
                nc.sync.dma_start(wq16[o][:], packed(wq_ins[o], P))
            nc.scalar.dma_start(xq8[1][:], packed(xq8_ins[1], 512))
            nc.scalar.dma_start(xq16[1][:], packed(xq_ins[1], 512))
            wk8 = rp.tile([P, ND, D], F8, tag="wk8")
            nc.sync.dma_start(wk8[:], packed(wk_in, D))
            x8t = rp.tile([P, ND, S], F8, tag="x8t")
            nc.scalar.dma_start(x8t[:], packed(x8_in, S))
            wv16 = rp.tile([P, ND, D], F16, tag="wv16")
            nc.sync.dma_start(wv16[:], packed(wv_in, D))
            xt16 = rp.tile([P, ND, S], F16, tag="xt16")
            nc.scalar.dma_start(xt16[:], packed(xT16_in, S))

            kT16 = rp.tile([P, NO, S], F16, tag="kT16")
            qT16 = rp.tile([P, NO, N_IB * IB], F16, tag="qT16")
            v_tiles = [
                rp.tile([P, D], F16, tag=f"v{j}", name=f"v{j}")
                for j in range(S // P)
            ]

            # ---- PE warm-up on an on-chip memset tile: no DMA dependency,
            # so the HAM ramp starts right after the framework preamble and
            # covers the ~7us DMA cold-start of the first real loads.
            warm16 = rp.tile([P, 512], F16, tag="warm16")
            nc.vector.memset(warm16[:], 1.0)
            psA_cm = tc.tile_pool(name="psA", bufs=2, space="PSUM")
            psA = psA_cm.__enter__()
            wps = psA.tile([P, 512], F32, tag="wps", name="wps", bufs=1)
            for w in range(N_WARM):
                _mm(nc, wps[:], warm16[:, 0:P], warm16[:], start=True, stop=True)

            # ---------------- Phase A: projections ----------------
            # Q: fp16, psum [o 128, i 512]; store as fp8 for the scores mm
            for sb in range(2):
                for o in range(NO):
                    pq = psA.tile([P, 512], F32, tag="pp", name=f"pq{sb}_{o}")
                    for g in range(3):
                        nc.tensor.matmul(
                            pq[:], wq8[o][:, 2 * g:2 * g + 2, :],
                            xq8[sb][:, 2 * g:2 * g + 2, :],
                            start=(g == 0), stop=False, perf_mode=DR,
                            skip_group_check=True)
                    for d in range(2):
                        nc.tensor.matmul(
                            pq[:], wq16[o][:, d, :], xq16[sb][:, d, :],
                            start=False, stop=(d == 1),
                            skip_group_check=True)
                    nc.vector.tensor_copy(
                        qT16[:, o, sb * 512:(sb + 1) * 512], pq[:])

            # K: fp8 DoubleRow, psum [o 128, j 512]; store fp8
            for jb in range(S // 512):
                for o in range(NO):
                    pk = psA.tile([P, 512], F32, tag="pp", name=f"pk{jb}_{o}")
                    for g in range(ND // 2):
                        _mm(nc, pk[:],
                            wk8[:, 2 * g:2 * g + 2, o * P:(o + 1) * P],
                            x8t[:, 2 * g:2 * g + 2, jb * 512:(jb + 1) * 512],
                            start=(g == 0), stop=(g == ND // 2 - 1),
                            perf_mode=DR)
                    nc.vector.tensor_copy(
                        kT16[:, o, jb * 512:(jb + 1) * 512], pk[:])

            # V: fp16, psum [j 128, o 512]; x tile stationary, wv moving
            for jb in range(S // 512):
                for jj in range(4):
                    jt = jb * 4 + jj
                    for ob in range(2):
                        pv = psA.tile([P, 512], F32, tag="pp", name=f"pv{jt}_{ob}")
                        for d in range(ND):
                            _mm(nc, pv[:],
                                xt16[:, d, jt * P:(jt + 1) * P],
                                wv16[:, d, ob * 512:(ob + 1) * 512],
                                start=(d == 0), stop=(d == ND - 1))
                        nc.vector.tensor_copy(
                            v_tiles[jt][:, ob * 512:(ob + 1) * 512], pv[:])

            psA_cm.__exit__(None, None, None)

            # ---------------- Phase B: attention ----------------
            with (
                tc.tile_pool(name="ex", bufs=3) as expool,
                tc.tile_pool(name="ost", bufs=2) as ostpool,
                tc.tile_pool(name="rcp", bufs=4) as rcpool,
                tc.tile_pool(name="psS", bufs=2, space="PSUM") as psS,
                tc.tile_pool(name="psC", bufs=1, space="PSUM") as psC,
                tc.tile_pool(name="psD", bufs=1, space="PSUM") as psD,
            ):
                def emit_scores(s, t, ps):
                    for o in range(NO):
                        _mm(nc, ps[:],
                            kT16[:, o, t * P:(t + 1) * P],
                            qT16[:, o, s * IB:(s + 1) * IB],
                            start=(o == 0), stop=(o == NO - 1))

                for s in reversed(range(N_IB)):
                    jt_n = JT_SLOTS[s]
                    cps = [
                        [
                            psC.tile([P, 512], F32, tag=f"c{it}{ob}",
                                     name=f"c{s}_{it}{ob}")
                            for ob in range(2)
                        ]
                        for it in range(2)
                    ]
                    dps = [
                        psD.tile([P, 2], F32, tag=f"d{it}", name=f"d{s}_{it}")
                        for it in range(2)
                    ]
                    ps_t = [None] * jt_n
                    ps_t[0] = psS.tile([P, IB], F32, tag="ps", name=f"ps{s}_0")
                    emit_scores(s, 0, ps_t[0])
                    for t in range(jt_n):
                        if t + 1 < jt_n:
                            ps_t[t + 1] = psS.tile([P, IB], F32, tag="ps",
                                                   name=f"ps{s}_{t + 1}")
                            emit_scores(s, t + 1, ps_t[t + 1])
                        ps = ps_t[t]
                        et = expool.tile([P, IB], F16, tag="et", name=f"et{s}_{t}")
                        if t >= jt_n - 4:
                            eraw = expool.tile([P, IB], F16, tag="eraw",
                                               name=f"er{s}_{t}")
                            nc.scalar.activation(
                                eraw[:], ps[:],
                                mybir.ActivationFunctionType.Exp, scale=scale,
                            )
                            col = s * 4 + (t - (jt_n - 4))
                            nc.vector.scalar_tensor_tensor(
                                et[:], t0_t[:], delta_t[:, col:col + 1], eraw[:],
                                op0=mybir.AluOpType.is_le,
                                op1=mybir.AluOpType.mult,
                            )
                        else:
                            nc.scalar.activation(
                                et[:], ps[:],
                                mybir.ActivationFunctionType.Exp, scale=scale,
                            )
                        last = t == jt_n - 1
                        for it in range(2):
                            lhs = et[:, it * P:(it + 1) * P]
                            for ob in range(2):
                                _mm(nc, cps[it][ob][:], lhs,
                                    v_tiles[t][:, ob * 512:(ob + 1) * 512],
                                    start=(t == 0), stop=last)
                            _mm(nc, dps[it][:], lhs, ones_t[:],
                                start=(t == 0), stop=last)
                    for it in range(2):
                        rc = rcpool.tile([P, 1], F32, tag="rc", name=f"rc{s}_{it}")
                        nc.vector.reciprocal(rc[:], dps[it][:, 0:1])
                        ot = ostpool.tile([P, D], F16, tag="ot", name=f"ot{s}_{it}")
                        rows = slice(s * IB + it * P, s * IB + (it + 1) * P)
                        for ob in range(2):
                            cols = slice(ob * 512, (ob + 1) * 512)
                            nc.vector.tensor_scalar_mul(
                                ot[:, cols], cps[it][ob][:], rc[:]
                            )
                            nc.sync.dma_start(out[rows, cols], ot[:, cols])

    nc.compile()
    return nc


_NC_CACHE = None


def _get_nc():
    global _NC_CACHE
    if _NC_CACHE is None:
        _NC_CACHE = build_program()
    return _NC_CACHE


def _pack(a2d):
    """[ND*P, C] d-major -> [P, ND*C] partition-major (SBUF layout)."""
    d, c = a2d.shape
    return np.ascontiguousarray(
        a2d.reshape(ND, P, c).transpose(1, 0, 2).reshape(P, ND * c))


def make_core_inputs(x, Wq, Wk, Wv):
    """Host-side shard prep. Returns list of 8 in_maps."""
    x = np.asarray(x, dtype=np.float32)
    wqT_f = np.asarray(Wq, np.float32).T.astype(np.float16)

    def _packn(a2d, nd):
        d, c = a2d.shape
        return np.ascontiguousarray(
            a2d.reshape(nd, P, c).transpose(1, 0, 2).reshape(P, nd * c))

    wq8_slices = [_packn(np.ascontiguousarray(
        wqT_f[0:768, o * P:(o + 1) * P]).astype(ml_dtypes.float8_e4m3), 6)
        for o in range(8)]
    wq_slices = [_packn(np.ascontiguousarray(
        wqT_f[768:1024, o * P:(o + 1) * P]), 2) for o in range(8)]
    wkT = _pack(np.asarray(Wk, np.float32).T.astype(ml_dtypes.float8_e4m3))
    wvT = _pack(np.asarray(Wv, np.float32).T.astype(np.float16))
    t0 = (np.arange(P, dtype=np.float32)[:, None]
          - np.arange(IB, dtype=np.float32)[None, :]).astype(np.float16)
    t0 = np.ascontiguousarray(t0)

    in_maps = []
    for c in range(N_CORES):
        b, r = divmod(c, 2)
        starts = ROLE_STARTS[r]
        xT = np.ascontiguousarray(x[b].T)
        xq = np.concatenate([x[b][i0:i0 + IB, :] for i0 in starts], axis=0)
        xqT_f = xq.T.astype(np.float16)
        xq8c = [_packn(np.ascontiguousarray(
            xqT_f[0:768, i * 512:(i + 1) * 512]).astype(ml_dtypes.float8_e4m3), 6)
            for i in range(2)]
        xq16c = [_packn(np.ascontiguousarray(
            xqT_f[768:1024, i * 512:(i + 1) * 512]), 2) for i in range(2)]
        delta = np.empty((P, 16), np.float16)
        for s in range(N_IB):
            for tr in range(4):
                t = JT_SLOTS[s] - 4 + tr
                delta[:, s * 4 + tr] = float(starts[s] - P * t)
        in_maps.append({
            "xT16": _pack(xT.astype(np.float16)),
            "x8": _pack(xT.astype(ml_dtypes.float8_e4m3)),
            "xqT0": xq16c[0], "xqT1": xq16c[1],
            "xq8T0": xq8c[0], "xq8T1": xq8c[1],
            **{f"wqT{o}": wq_slices[o] for o in range(8)},
            **{f"wq8T{o}": wq8_slices[o] for o in range(8)},
            "wkT": wkT, "wvT": wvT,
            "t0": t0, "delta": np.ascontiguousarray(delta),
            "ones": np.ones((P, 2), np.float16),
        })
    return in_maps


def assemble_output(results):
    """Gather 8 per-core [1024, 1024] outputs into [B, S, D]."""
    out = np.empty((B, S, D), np.float32)
    for c in range(N_CORES):
        b, r = divmod(c, 2)
        starts = ROLE_STARTS[r]
        oc = results[c]["out"]
        for s, i0 in enumerate(starts):
            out[b, i0:i0 + IB, :] = oc[s * IB:(s + 1) * IB, :].astype(np.float32)
    return out


def kernel(x, Wq, Wk, Wv):
    nc = _get_nc()
    in_maps = make_core_inputs(x, Wq, Wk, Wv)
    res = run_bass_kernel_spmd(nc, in_maps, list(range(N_CORES)))
    return assemble_output(res.results)


# revision 23
# speedup vs baseline: 1.0521x; 1.0521x over previous
"""
Causal self-attention (single head) on 8 trn2 NeuronCores.

Problem: x[4, 2048, 1024], Wq/Wk/Wv[1024, 1024] (torch Linear layout [d_out, d_in]).
    q/k/v = x @ W.T ; out = softmax(mask(q k^T) / 32) @ v

Sharding (no collectives, uniform SPMD program):
  core c -> batch b = c // 2, role r = c % 2.
  Both cores of a pair compute K/V projections for the full 2048-row
  sequence of their batch (duplicated work, avoids cross-core comms).
  Query rows are split between the pair in 4 i-blocks of 256 rows with
  per-slot padded causal extents JT_SLOTS = [4, 8, 12, 16] identical for
  both roles; causality inside the padded slots is enforced with a
  per-core "delta" input (keep iff jj - ii <= delta).

Precision plan (error gate is 2e-2 relative; measured 1.45e-2):
  fp16 is the working dtype (all |values| well inside fp16 range).
  K-projection runs as fp8e4m3 DoubleRow matmuls (2 weights/PE cell,
  2x MACs/cycle, measured 222ns per 256-deep x 512-wide MM vs 2x217ns
  in fp16); q/k are stored fp16 and scores run fp16 (fp8 q/k stores +
  fp8 scores measured 2.4e-2 -- over the gate). V path stays fp16.
  The output DMAs as fp16 and is widened to fp32 on the host.

Everything is SBUF-resident (~170 KB of the 208 KB/partition): x in
fp16 + fp8, all three weights, kT16/qT16, and the 16 v tiles. No DRAM
spills; the only DMA is inputs in (~11 MB) and the output (2 MB).
Inputs are host-packed to the SBUF partition-major layout so each load
is 128 contiguous runs (sliced d-major views cost ~11ns/descriptor x
1024 descriptors of engine issue time per load, and the transfers ran
at half bandwidth). The two tensors the first Q psum groups read ship
as their own contiguous tensors and load first; PE warm-up matmuls on
a memset tile bridge the DMA cold start (~6.5us framework preamble +
~11us first-wave DMA) while holding the HAM clock gate at 8/8.

Phase B is software-pipelined: scores(t+1) is issued to the PE before
ctx(t), so the activation-engine exp(t) latency hides behind the next
scores matmul and the PE never idles. Measured PE idle inside the
kernel body: ~1-4us total.
"""

import sys

for _p in ("/opt/trn_rl_repo", "/root/.axon_site/_ro/trn_rl_repo"):
    if _p not in sys.path:
        sys.path.append(_p)

import numpy as np
import ml_dtypes

import concourse.bass as bass
import concourse.mybir as mybir
import concourse.tile as tile
from concourse import bacc
from concourse.bass_utils import run_bass_kernel_spmd
import concourse.bass_utils as _bu

# NOTE: walrus --enable-ldw-opt stays at its default (false): the measured
# back-to-back MM rate is full speed without it for fp16/fp8 (216.5 ns per
# 512-row MM), and enabling it rejects DoubleRow LDWEIGHTS instructions.

F32 = mybir.dt.float32
F16 = mybir.dt.float16
F8 = mybir.dt.float8e4
DR = mybir.MatmulPerfMode.DoubleRow

B, S, D = 4, 2048, 1024
P = 128
ND = D // P          # 8 d-tiles (projection contraction)
NO = D // P          # 8 o-tiles
IB = 256             # i-block (query block) rows
N_IB = 4
JT_SLOTS = [4, 8, 12, 16]
ROLE_STARTS = {
    0: [0, 768, 1024, 1792],
    1: [256, 512, 1280, 1536],
}
N_CORES = 8
N_WARM = 30


def _mm(nc, out, lhsT, rhs, start, stop, perf_mode=None):
    nc.tensor.matmul(out, lhsT, rhs, start=start, stop=stop,
                     perf_mode=perf_mode)


def build_program():
    nc = bacc.Bacc(
        "TRN2",
        target_bir_lowering=False,
        debug=False,
        enable_asserts=False,
        num_devices=N_CORES,
    )
    # Inputs are host-packed to the SBUF partition-major layout
    # [P, ND*C]: each DMA is 128 contiguous 16KB-ish runs instead of 1024
    # 1KB runs -- descriptor generation (~11ns each) was costing 5-11us of
    # engine issue time per load the d-major way.
    xT16_in = nc.dram_tensor("xT16", [P, ND * S], F16, kind="ExternalInput").ap()
    x8_in = nc.dram_tensor("x8", [P, ND * S], F8, kind="ExternalInput").ap()
    # wq/xq ship as per-chunk contiguous tensors: the first Q psum groups
    # need only wq half 0 + xq chunk 0 (2MB), and a fully-contiguous
    # [P, 8K] DMA has 128 descriptors vs 1024 for a sliced view.
    # Q runs half-fp8: contraction d-tiles 0..3 as fp8 DoubleRow, 4..7 in
    # fp16 (full-fp8 q would push the scores error over the 2e-2 gate; the
    # half split lands ~1.9e-2 and shaves 7us off the Q projection).
    xq_ins = [nc.dram_tensor(f"xqT{i}", [P, 4 * 512], F16,
                             kind="ExternalInput").ap() for i in range(2)]
    xq8_ins = [nc.dram_tensor(f"xq8T{i}", [P, 4 * 512], F8,
                              kind="ExternalInput").ap() for i in range(2)]
    wq_ins = [nc.dram_tensor(f"wqT{i}", [P, 4 * P], F16,
                             kind="ExternalInput").ap() for i in range(8)]
    wq8_ins = [nc.dram_tensor(f"wq8T{i}", [P, 4 * P], F8,
                              kind="ExternalInput").ap() for i in range(8)]
    wk_in = nc.dram_tensor("wkT", [P, ND * D], F8, kind="ExternalInput").ap()
    wv_in = nc.dram_tensor("wvT", [P, ND * D], F16, kind="ExternalInput").ap()
    t0_in = nc.dram_tensor("t0", [P, IB], F16, kind="ExternalInput").ap()
    delta_in = nc.dram_tensor("delta", [P, 16], F16, kind="ExternalInput").ap()
    ones_in = nc.dram_tensor("ones", [P, 2], F16, kind="ExternalInput").ap()
    out = nc.dram_tensor("out", [N_IB * IB, D], F16, kind="ExternalOutput").ap()

    scale = 1.0 / 32.0  # 1/sqrt(d_v)

    def packed(ap2d, c):
        # [P, ND*C] host-packed DRAM view -> [P, ND, C] 3D AP
        return ap2d.rearrange("p (nd c) -> p nd c", c=c)

    with tile.TileContext(nc) as tc:
        with tc.tile_pool(name="res", bufs=1) as rp:
            # ---- constants first (warm-up stationary/moving data) ----
            t0_t = rp.tile([P, IB], F16, tag="t0")
            nc.gpsimd.dma_start(t0_t[:], t0_in[:])
            delta_t = rp.tile([P, 16], F16, tag="delta")
            nc.gpsimd.dma_start(delta_t[:], delta_in[:])
            ones_t = rp.tile([P, 2], F16, tag="ones")
            nc.gpsimd.dma_start(ones_t[:], ones_in[:])

            # ---- resident tensors + their loads ----
            # Two DMA queues, each FIFO-serialized in consumption order so
            # the Q-projection inputs get the full HBM bandwidth first
            # (concurrent queues split it and delayed Q's start by ~11us).
            # First wave is only the 2MB the first Q psum groups need
            # (wq o-half 0 + xq chunk 0), so Q starts ~6us earlier; the
            # rest streams behind it in consumption order, per-queue FIFO.
            xq16 = [rp.tile([P, 4, 512], F16, tag=f"xq16_{i}",
                            name=f"xq16_{i}") for i in range(2)]
            xq8 = [rp.tile([P, 4, 512], F8, tag=f"xq8_{i}",
                           name=f"xq8_{i}") for i in range(2)]
            wq16 = [rp.tile([P, 4, P], F16, tag=f"wq16_{i}",
                            name=f"wq16_{i}") for i in range(8)]
            wq8 = [rp.tile([P, 4, P], F8, tag=f"wq8_{i}",
                           name=f"wq8_{i}") for i in range(8)]
            nc.scalar.dma_start(xq8[0][:], packed(xq8_ins[0], 512))
            nc.scalar.dma_start(xq16[0][:], packed(xq_ins[0], 512))
            for o in range(8):
                nc.sync.dma_start(wq8[o][:], packed(wq8_ins[o], P))
                nc.sync.dma_start(wq16[o][:], packed(wq_ins[o], P))
            nc.scalar.dma_start(xq8[1][:], packed(xq8_ins[1], 512))
            nc.scalar.dma_start(xq16[1][:], packed(xq_ins[1], 512))
            wk8 = rp.tile([P, ND, D], F8, tag="wk8")
            nc.sync.dma_start(wk8[:], packed(wk_in, D))
            x8t = rp.tile([P, ND, S], F8, tag="x8t")
            nc.scalar.dma_start(x8t[:], packed(x8_in, S))
            wv16 = rp.tile([P, ND, D], F16, tag="wv16")
            nc.sync.dma_start(wv16[:], packed(wv_in, D))
            xt16 = rp.tile([P, ND, S], F16, tag="xt16")
            nc.scalar.dma_start(xt16[:], packed(xT16_in, S))

            kT16 = rp.tile([P, NO, S], F16, tag="kT16")
            qT16 = rp.tile([P, NO, N_IB * IB], F16, tag="qT16")
            v_tiles = [
                rp.tile([P, D], F16, tag=f"v{j}", name=f"v{j}")
                for j in range(S // P)
            ]

            # ---- PE warm-up on an on-chip memset tile: no DMA dependency,
            # so the HAM ramp starts right after the framework preamble and
            # covers the ~7us DMA cold-start of the first real loads.
            warm16 = rp.tile([P, 512], F16, tag="warm16")
            nc.vector.memset(warm16[:], 1.0)
            psA_cm = tc.tile_pool(name="psA", bufs=2, space="PSUM")
            psA = psA_cm.__enter__()
            wps = psA.tile([P, 512], F32, tag="wps", name="wps", bufs=1)
            for w in range(N_WARM):
                _mm(nc, wps[:], warm16[:, 0:P], warm16[:], start=True, stop=True)

            # ---------------- Phase A: projections ----------------
            # Q: fp16, psum [o 128, i 512]; store as fp8 for the scores mm
            for sb in range(2):
                for o in range(NO):
                    pq = psA.tile([P, 512], F32, tag="pp", name=f"pq{sb}_{o}")
                    for g in range(2):
                        nc.tensor.matmul(
                            pq[:], wq8[o][:, 2 * g:2 * g + 2, :],
                            xq8[sb][:, 2 * g:2 * g + 2, :],
                            start=(g == 0), stop=False, perf_mode=DR,
                            skip_group_check=True)
                    for d in range(4):
                        nc.tensor.matmul(
                            pq[:], wq16[o][:, d, :], xq16[sb][:, d, :],
                            start=False, stop=(d == 3),
                            skip_group_check=True)
                    nc.vector.tensor_copy(
                        qT16[:, o, sb * 512:(sb + 1) * 512], pq[:])

            # K: fp8 DoubleRow, psum [o 128, j 512]; store fp8
            for jb in range(S // 512):
                for o in range(NO):
                    pk = psA.tile([P, 512], F32, tag="pp", name=f"pk{jb}_{o}")
                    for g in range(ND // 2):
                        _mm(nc, pk[:],
                            wk8[:, 2 * g:2 * g + 2, o * P:(o + 1) * P],
                            x8t[:, 2 * g:2 * g + 2, jb * 512:(jb + 1) * 512],
                            start=(g == 0), stop=(g == ND // 2 - 1),
                            perf_mode=DR)
                    nc.vector.tensor_copy(
                        kT16[:, o, jb * 512:(jb + 1) * 512], pk[:])

            # V: fp16, psum [j 128, o 512]; x tile stationary, wv moving
            for jb in range(S // 512):
                for jj in range(4):
                    jt = jb * 4 + jj
                    for ob in range(2):
                        pv = psA.tile([P, 512], F32, tag="pp", name=f"pv{jt}_{ob}")
                        for d in range(ND):
                            _mm(nc, pv[:],
                                xt16[:, d, jt * P:(jt + 1) * P],
                                wv16[:, d, ob * 512:(ob + 1) * 512],
                                start=(d == 0), stop=(d == ND - 1))
                        nc.vector.tensor_copy(
                            v_tiles[jt][:, ob * 512:(ob + 1) * 512], pv[:])

            psA_cm.__exit__(None, None, None)

            # ---------------- Phase B: attention ----------------
            with (
                tc.tile_pool(name="ex", bufs=3) as expool,
                tc.tile_pool(name="ost", bufs=2) as ostpool,
                tc.tile_pool(name="rcp", bufs=4) as rcpool,
                tc.tile_pool(name="psS", bufs=2, space="PSUM") as psS,
                tc.tile_pool(name="psC", bufs=1, space="PSUM") as psC,
                tc.tile_pool(name="psD", bufs=1, space="PSUM") as psD,
            ):
                def emit_scores(s, t, ps):
                    for o in range(NO):
                        _mm(nc, ps[:],
                            kT16[:, o, t * P:(t + 1) * P],
                            qT16[:, o, s * IB:(s + 1) * IB],
                            start=(o == 0), stop=(o == NO - 1))

                for s in reversed(range(N_IB)):
                    jt_n = JT_SLOTS[s]
                    cps = [
                        [
                            psC.tile([P, 512], F32, tag=f"c{it}{ob}",
                                     name=f"c{s}_{it}{ob}")
                            for ob in range(2)
                        ]
                        for it in range(2)
                    ]
                    dps = [
                        psD.tile([P, 2], F32, tag=f"d{it}", name=f"d{s}_{it}")
                        for it in range(2)
                    ]
                    ps_t = [None] * jt_n
                    ps_t[0] = psS.tile([P, IB], F32, tag="ps", name=f"ps{s}_0")
                    emit_scores(s, 0, ps_t[0])
                    for t in range(jt_n):
                        if t + 1 < jt_n:
                            ps_t[t + 1] = psS.tile([P, IB], F32, tag="ps",
                                                   name=f"ps{s}_{t + 1}")
                            emit_scores(s, t + 1, ps_t[t + 1])
                        ps = ps_t[t]
                        et = expool.tile([P, IB], F16, tag="et", name=f"et{s}_{t}")
                        if t >= jt_n - 4:
                            eraw = expool.tile([P, IB], F16, tag="eraw",
                                               name=f"er{s}_{t}")
                            nc.scalar.activation(
                                eraw[:], ps[:],
                                mybir.ActivationFunctionType.Exp, scale=scale,
                            )
                            col = s * 4 + (t - (jt_n - 4))
                            nc.vector.scalar_tensor_tensor(
                                et[:], t0_t[:], delta_t[:, col:col + 1], eraw[:],
                                op0=mybir.AluOpType.is_le,
                                op1=mybir.AluOpType.mult,
                            )
                        else:
                            nc.scalar.activation(
                                et[:], ps[:],
                                mybir.ActivationFunctionType.Exp, scale=scale,
                            )
                        last = t == jt_n - 1
                        for it in range(2):
                            lhs = et[:, it * P:(it + 1) * P]
                            for ob in range(2):
                                _mm(nc, cps[it][ob][:], lhs,
                                    v_tiles[t][:, ob * 512:(ob + 1) * 512],
                                    start=(t == 0), stop=last)
                            _mm(nc, dps[it][:], lhs, ones_t[:],
                                start=(t == 0), stop=last)
                    for it in range(2):
                        rc = rcpool.tile([P, 1], F32, tag="rc", name=f"rc{s}_{it}")
                        nc.vector.reciprocal(rc[:], dps[it][:, 0:1])
                        ot = ostpool.tile([P, D], F16, tag="ot", name=f"ot{s}_{it}")
                        rows = slice(s * IB + it * P, s * IB + (it + 1) * P)
                        for ob in range(2):
                            cols = slice(ob * 512, (ob + 1) * 512)
                            nc.vector.tensor_scalar_mul(
                                ot[:, cols], cps[it][ob][:], rc[:]
                            )
                            nc.sync.dma_start(out[rows, cols], ot[:, cols])

    nc.compile()
    return nc


_NC_CACHE = None


def _get_nc():
    global _NC_CACHE
    if _NC_CACHE is None:
        _NC_CACHE = build_program()
    return _NC_CACHE


def _pack(a2d):
    """[ND*P, C] d-major -> [P, ND*C] partition-major (SBUF layout)."""
    d, c = a2d.shape
    return np.ascontiguousarray(
        a2d.reshape(ND, P, c).transpose(1, 0, 2).reshape(P, ND * c))


def make_core_inputs(x, Wq, Wk, Wv):
    """Host-side shard prep. Returns list of 8 in_maps."""
    x = np.asarray(x, dtype=np.float32)
    wqT_f = np.asarray(Wq, np.float32).T.astype(np.float16)

    def _pack4(a2d):
        d, c = a2d.shape
        return np.ascontiguousarray(
            a2d.reshape(4, P, c).transpose(1, 0, 2).reshape(P, 4 * c))

    wq8_slices = [_pack4(np.ascontiguousarray(
        wqT_f[0:512, o * P:(o + 1) * P]).astype(ml_dtypes.float8_e4m3))
        for o in range(8)]
    wq_slices = [_pack4(np.ascontiguousarray(
        wqT_f[512:1024, o * P:(o + 1) * P])) for o in range(8)]
    wkT = _pack(np.asarray(Wk, np.float32).T.astype(ml_dtypes.float8_e4m3))
    wvT = _pack(np.asarray(Wv, np.float32).T.astype(np.float16))
    t0 = (np.arange(P, dtype=np.float32)[:, None]
          - np.arange(IB, dtype=np.float32)[None, :]).astype(np.float16)
    t0 = np.ascontiguousarray(t0)

    in_maps = []
    for c in range(N_CORES):
        b, r = divmod(c, 2)
        starts = ROLE_STARTS[r]
        xT = np.ascontiguousarray(x[b].T)
        xq = np.concatenate([x[b][i0:i0 + IB, :] for i0 in starts], axis=0)
        xqT_f = xq.T.astype(np.float16)
        xq8c = [_pack4(np.ascontiguousarray(
            xqT_f[0:512, i * 512:(i + 1) * 512]).astype(ml_dtypes.float8_e4m3))
            for i in range(2)]
        xq16c = [_pack4(np.ascontiguousarray(
            xqT_f[512:1024, i * 512:(i + 1) * 512])) for i in range(2)]
        delta = np.empty((P, 16), np.float16)
        for s in range(N_IB):
            for tr in range(4):
                t = JT_SLOTS[s] - 4 + tr
                delta[:, s * 4 + tr] = float(starts[s] - P * t)
        in_maps.append({
            "xT16": _pack(xT.astype(np.float16)),
            "x8": _pack(xT.astype(ml_dtypes.float8_e4m3)),
            "xqT0": xq16c[0], "xqT1": xq16c[1],
            "xq8T0": xq8c[0], "xq8T1": xq8c[1],
            **{f"wqT{o}": wq_slices[o] for o in range(8)},
            **{f"wq8T{o}": wq8_slices[o] for o in range(8)},
            "wkT": wkT, "wvT": wvT,
            "t0": t0, "delta": np.ascontiguousarray(delta),
            "ones": np.ones((P, 2), np.float16),
        })
    return in_maps


def assemble_output(results):
    """Gather 8 per-core [1024, 1024] outputs into [B, S, D]."""
    out = np.empty((B, S, D), np.float32)
    for c in range(N_CORES):
        b, r = divmod(c, 2)
        starts = ROLE_STARTS[r]
        oc = results[c]["out"]
        for s, i0 in enumerate(starts):
            out[b, i0:i0 + IB, :] = oc[s * IB:(s + 1) * IB, :].astype(np.float32)
    return out


def kernel(x, Wq, Wk, Wv):
    nc = _get_nc()
    in_maps = make_core_inputs(x, Wq, Wk, Wv)
    res = run_bass_kernel_spmd(nc, in_maps, list(range(N_CORES)))
    return assemble_output(res.results)
